# revision 1
# baseline (speedup 1.0000x reference)
"""Cross-attention Trainium2 Bass kernel.

Reference computation (per batch b):
    q = relu(scale_q * (Wq @ qf) + bias_q)          [C, Nq]
    k = relu(scale_k * (Wk @ kf) + bias_k)          [C, Nk]
    v = relu(scale_v * (Wv @ kf) + bias_v)          [C, Nk]
    sim  = q.T @ k / sqrt(C)                        [Nq, Nk]
    attn = softmax(sim, axis=-1)
    ctx  = v @ attn.T                               [C, Nq]

Sharding: 8 cores = 4 batches x 2 query halves (Nq 4096 -> 2048 per core).
Each core gets the full K/V for its batch (recomputed, cheap) and half the
query positions; output halves are concatenated on the host.

Device-side design (per core):
  - BN scale folded into the weights on the host; weights fed pre-transposed.
  - All matmuls run in float32r (fp32 with 11-bit mantissa, full PE rate for
    free dims >= 256). DMA-fed matmul operands are pre-rounded on the host;
    on-device matmul operands are produced by ACT/DVE writing float32r tiles.
  - sim is computed transposed (k on partitions, q on free dim) so the
    exp(sim) tiles feed the ctx matmul as the moving operand directly -- no
    attention transpose is ever needed.
  - softmax uses a constant shift instead of a row max: exp(sim/sqrt(C) - 4)
    (sim/sqrt(C) is bounded by ~|q||k|/16 << 88, so no overflow is possible),
    and the row sums come from a 257th "ones" channel appended to v^T.
  - ctx is accumulated unnormalized; normalization multiplies by 1/sums
    broadcast across partitions via a K=1 matmul with a ones column.
"""

import sys

for _p in ("/opt/trn_rl_repo", "/root/.axon_site/_ro/trn_rl_repo"):
    if _p not in sys.path:
        sys.path.insert(0, _p)

import numpy as np

import concourse.bacc as bacc
import concourse.mybir as mybir
import concourse.tile as tile
from concourse.bass_utils import run_bass_kernel_spmd

F32 = mybir.dt.float32
F32R = mybir.dt.float32r
AF = mybir.ActivationFunctionType

B, C, H, W = 4, 256, 64, 64
NK = H * W          # 4096 key positions per batch
NQ = NK // 2        # 2048 query positions per core
P = 128
CO = C // P         # 2 contraction subtiles
QC = 512            # query chunk (matmul moving free dim)
NQC = NQ // QC      # 4 query chunks per core
KT = NK // P        # 32 key tiles
EXP_SHIFT = -4.0    # exp(sim/sqrt(C) + EXP_SHIFT); sim/sqrt(C) observed in [0.5, 7.5]
SCALE = 1.0 / np.sqrt(C)


def _round_fp32r(x: np.ndarray) -> np.ndarray:
    """Round fp32 to fp32r (11-bit mantissa, RNE) as the PE datapath expects."""
    u = np.ascontiguousarray(x, dtype=np.float32).view(np.uint32)
    lsb = (u >> 12) & 1
    r = ((u + 0x7FF + lsb) & np.uint32(0xFFFFF000)).astype(np.uint32)
    return r.view(np.float32)


def _build_program():
    nc = bacc.Bacc("TRN2", target_bir_lowering=False, debug=False)

    qf = nc.dram_tensor("qf", [C, NQ], F32R, kind="ExternalInput").ap()
    kf = nc.dram_tensor("kf", [C, NK], F32R, kind="ExternalInput").ap()
    wqT = nc.dram_tensor("wqT", [C, C], F32R, kind="ExternalInput").ap()
    wkT = nc.dram_tensor("wkT", [C, C], F32R, kind="ExternalInput").ap()
    wvT = nc.dram_tensor("wvT", [C, C + 2], F32R, kind="ExternalInput").ap()
    bq = nc.dram_tensor("bq", [P, CO], F32, kind="ExternalInput").ap()
    bk = nc.dram_tensor("bk", [P, CO], F32, kind="ExternalInput").ap()
    bvb = nc.dram_tensor("bvb", [P, C + 2], F32R, kind="ExternalInput").ap()
    ones = nc.dram_tensor("ones", [1, P], F32R, kind="ExternalInput").ap()
    out = nc.dram_tensor("out", [C, NQ], F32, kind="ExternalOutput").ap()
    out_t = out.rearrange("(co ci) n -> ci co n", ci=P)

    with tile.TileContext(nc) as tc:
        with (
            nc.allow_low_precision(reason="fp32r matmul operands (11-bit mantissa)"),
            tc.tile_pool(name="consts", bufs=1) as consts,
            tc.tile_pool(name="persist", bufs=1) as persist,
        ):
            # ---- constants (issue order matters: the first projection only
            # needs wqT + bq + the first qf chunk, so those go first and the
            # remaining weights ride behind the qf/kf streams) ----
            wqT_sb = consts.tile([P, CO, C], F32R, name="wqT_sb")
            nc.gpsimd.dma_start(wqT_sb[:], wqT.rearrange("(co ci) o -> ci co o", ci=P))
            bq_sb = consts.tile([P, CO], F32, name="bq_sb")
            wkT_sb = consts.tile([P, CO, C], F32R, name="wkT_sb")
            wvT_sb = consts.tile([P, CO, C + 2], F32R, name="wvT_sb")
            bk_sb = consts.tile([P, CO], F32, name="bk_sb")
            bvb_sb = consts.tile([P, C + 2], F32R, name="bvb_sb")
            ones_sb = consts.tile([1, P], F32R, name="ones_sb")
            b0_sb = consts.tile([P, 1], F32, name="b0_sb")
            nc.vector.memset(b0_sb[:], EXP_SHIFT)
            # dummy activation: pulls the ~1.3us LoadActFuncSet into the
            # initial DMA-wait window instead of blocking the first relu
            warm_sb = consts.tile([P, 1], F32, name="warm_sb")
            nc.scalar.activation(warm_sb[:], b0_sb[:], AF.Relu)

            # ---- persistent activations ----
            q_sb = persist.tile([P, CO, NQ], F32R, name="q_sb")
            k_sb = persist.tile([P, CO, NK], F32R, name="k_sb")
            vT_sb = persist.tile([P, KT, C + 2], F32R, name="vT_sb")

            # ---- projections (staging pool scoped so its SBUF is reused) ----
            with (
                tc.tile_pool(name="staging", bufs=1) as staging,
                tc.tile_pool(name="proj_ps", bufs=1, space="PSUM") as proj_ps,
            ):
                # Input DMA plan. Each dma_start costs ~650ns of serial SP
                # dispatch, so: few DMAs, a small first chunk so the first
                # matmul starts ~2.5us in, and strictly need-before order.
                qf_sb = staging.tile([P, CO, NQ], F32R, name="qf_sb")
                qf_t = qf.rearrange("(co ci) n -> ci co n", ci=P)
                kf_sb = staging.tile([P, CO, NK], F32R, name="kf_sb")
                kf_t = kf.rearrange("(co ci) n -> ci co n", ci=P)
                nc.gpsimd.dma_start(bq_sb[:], bq[:])
                nc.gpsimd.dma_start(wkT_sb[:], wkT.rearrange("(co ci) o -> ci co o", ci=P))
                nc.gpsimd.dma_start(bk_sb[:], bk[:])
                nc.gpsimd.dma_start(wvT_sb[:], wvT.rearrange("(co ci) o -> ci co o", ci=P))
                nc.gpsimd.dma_start(bvb_sb[:], bvb[:])
                nc.gpsimd.dma_start(ones_sb[:], ones[:])
                nc.sync.dma_start(qf_sb[:, :, :QC], qf_t[:, :, :QC])
                nc.sync.dma_start(qf_sb[:, :, QC:], qf_t[:, :, QC:])
                nc.sync.dma_start(kf_sb[:, :, :2 * QC], kf_t[:, :, :2 * QC])
                nc.sync.dma_start(kf_sb[:, :, 2 * QC:5 * QC],
                                  kf_t[:, :, 2 * QC:5 * QC])
                nc.sync.dma_start(kf_sb[:, :, 5 * QC:], kf_t[:, :, 5 * QC:])

                def proj_iter(j, w_sb, bias_sb, dst, src_sb):
                    # one [*, QC] chunk of a q/k projection; relu+bias for
                    # oo=0 runs on ACT, oo=1 on DVE so neither engine
                    # rate-limits PE
                    for oo in range(CO):
                        ps = proj_ps.tile([P, QC], F32, tag="pj", bufs=2,
                                          name=f"ps_{j}_{oo}")
                        for co in range(CO):
                            nc.tensor.matmul(
                                ps[:],
                                w_sb[:, co, oo * P:(oo + 1) * P],
                                src_sb[:, co, j * QC:(j + 1) * QC],
                                start=(co == 0), stop=(co == CO - 1),
                            )
                        if oo == 0:
                            nc.scalar.activation(
                                dst[:, oo, j * QC:(j + 1) * QC], ps[:], AF.Relu,
                                bias=bias_sb[:, oo:oo + 1],
                            )
                        else:
                            nc.vector.tensor_scalar(
                                dst[:, oo, j * QC:(j + 1) * QC], ps[:],
                                bias_sb[:, oo:oo + 1], 0.0,
                                mybir.AluOpType.add, mybir.AluOpType.max,
                            )

                def vt_pair(kp):
                    # vT = relu(kf.T @ Wv'.T + bias_v): [n, o], n on partitions;
                    # column C is the ones channel (0-weight col + bias 1.0).
                    # Each half padded to a full bank. bias_v varies along the
                    # free dim here, so it is added from a host-broadcast tile
                    # on DVE, then relu on ACT -- no PE bias matmul needed.
                    psv = proj_ps.tile([P, 2, QC], F32, tag="pv", bufs=3,
                                       name=f"psv_{kp}")
                    for half in range(2):
                        kt = 2 * kp + half
                        for co in range(CO):
                            nc.tensor.matmul(
                                psv[:, half, :C + 2],
                                kf_sb[:, co, kt * P:(kt + 1) * P],
                                wvT_sb[:, co, :],
                                start=(co == 0), stop=(co == CO - 1),
                            )
                    vtmp = staging.tile([P, 2, C + 2], F32, tag="vtmp", bufs=3,
                                        name=f"vtmp_{kp}")
                    nc.vector.tensor_tensor(
                        vtmp[:], psv[:, :, :C + 2],
                        bvb_sb[:, None, :].to_broadcast((P, 2, C + 2)),
                        mybir.AluOpType.add,
                    )
                    nc.scalar.activation(
                        vT_sb[:, 2 * kp:2 * kp + 2, :], vtmp[:], AF.Relu)

                # q = relu(Wq' @ qf + bq): [o, n] with o on partitions.
                # j-major so each arriving qf/kf chunk is fully consumed at
                # once; the vT pairs for chunk j of kf ride along with proj-k
                # so PE work fills the relu latency.
                for j in range(NQ // QC):
                    proj_iter(j, wqT_sb, bq_sb, q_sb, qf_sb)
                for j in range(NK // QC):
                    proj_iter(j, wkT_sb, bk_sb, k_sb, kf_sb)
                    vt_pair(2 * j)
                    vt_pair(2 * j + 1)

            # ---- attention ----
            with (
                tc.tile_pool(name="expp", bufs=1) as expp,
                tc.tile_pool(name="outp", bufs=1) as outp,
                tc.tile_pool(name="attn_ps", bufs=1, space="PSUM") as attn_ps,
            ):
                # Software pipeline: step s emits sim+exp for chunk s
                # interleaved (at k-pair granularity) with the ctx/sums
                # matmuls consuming chunk s-1's exp tiles. PE's ctx work fills
                # the ACT-exp latency that otherwise stalls the sim phase, and
                # the per-chunk phase boundaries disappear.
                NP = KT // 2           # k-pairs per chunk
                exp_pairs = {}         # qc -> list of pair tiles

                def emit_sim_pair(qc, kp):
                    qs = slice(qc * QC, (qc + 1) * QC)
                    ps = attn_ps.tile([P, 2, QC], F32, tag="sim", bufs=2,
                                      name=f"pss_{qc}_{kp}")
                    for half in range(2):
                        kt = 2 * kp + half
                        for co in range(CO):
                            nc.tensor.matmul(
                                ps[:, half, :],
                                k_sb[:, co, kt * P:(kt + 1) * P],
                                q_sb[:, co, qs],
                                start=(co == 0), stop=(co == CO - 1),
                            )
                    et = expp.tile([P, 2, QC], F32R, tag="expT", bufs=20,
                                   name=f"expT_{qc}_{kp}")
                    nc.scalar.activation(et[:], ps[:], AF.Exp,
                                         bias=b0_sb[:], scale=float(SCALE))
                    exp_pairs.setdefault(qc, []).append(et)

                def emit_ctx_pair(qc, kp, ctx_ps, sums_ps):
                    # sums first so the reciprocal chain starts a few matmuls
                    # before the last ctx matmul retires
                    for half in range(2):
                        kt = 2 * kp + half
                        e = exp_pairs[qc][kp][:, half, :]
                        nc.tensor.matmul(
                            sums_ps[:],
                            vT_sb[:, kt, C:C + 2],
                            e,
                            start=(kt == 0), stop=(kt == KT - 1),
                            skip_group_check=True,
                        )
                        for ct in range(CO):
                            nc.tensor.matmul(
                                ctx_ps[ct][:],
                                vT_sb[:, kt, ct * P:(ct + 1) * P],
                                e,
                                start=(kt == 0), stop=(kt == KT - 1),
                                skip_group_check=True,
                            )

                def emit_last_pair_and_norm(qc, ctx_ps, sums_ps):
                    # Last k-pair of a chunk: finish the sums accumulation
                    # first, hoist the recip -> broadcast -> copy chain so it
                    # overlaps the remaining ctx matmuls, then the final muls
                    # + output DMAs only wait on the last ctx matmul.
                    qs = slice(qc * QC, (qc + 1) * QC)
                    kp = NP - 1
                    halves = [(2 * kp + h, exp_pairs[qc][kp][:, h, :])
                              for h in range(2)]
                    for kt, e in halves:
                        nc.tensor.matmul(
                            sums_ps[:], vT_sb[:, kt, C:C + 2], e,
                            start=(kt == 0), stop=(kt == KT - 1),
                            skip_group_check=True,
                        )
                    recip = outp.tile([1, QC], F32R, tag="recip", bufs=2,
                                      name=f"recip_{qc}")
                    nc.vector.reciprocal(recip[:], sums_ps[0:1, :])
                    bc_ps = attn_ps.tile([P, QC], F32, tag="sim", bufs=2,
                                         name=f"psb_{qc}")
                    nc.tensor.matmul(bc_ps[:], ones_sb[:], recip[:],
                                     start=True, stop=True)
                    bc_sb = outp.tile([P, QC], F32, tag="bc", bufs=2,
                                      name=f"bc_{qc}")
                    nc.vector.tensor_copy(out=bc_sb[:], in_=bc_ps[:])
                    for kt, e in halves:
                        for ct in range(CO):
                            nc.tensor.matmul(
                                ctx_ps[ct][:],
                                vT_sb[:, kt, ct * P:(ct + 1) * P],
                                e,
                                start=(kt == 0), stop=(kt == KT - 1),
                                skip_group_check=True,
                            )
                    for ct in range(CO):
                        ot = outp.tile([P, QC], F32, tag="out", bufs=3,
                                       name=f"out_{qc}_{ct}")
                        nc.vector.tensor_mul(ot[:], ctx_ps[ct][:], bc_sb[:])
                        nc.sync.dma_start(out_t[:, ct, qs], ot[:])

                ctx_live = None  # (qc, ctx_ps, sums_ps) being accumulated
                for s in range(NQC + 1):
                    if s > 0:
                        qcp = s - 1
                        ctx_ps = [
                            attn_ps.tile([P, QC], F32, tag="ctx", bufs=2,
                                         name=f"psc_{qcp}_{ct}")
                            for ct in range(CO)
                        ]
                        sums_ps = attn_ps.tile([2, QC], F32, tag="sums", bufs=1,
                                               name=f"psS_{qcp}")
                        ctx_live = (qcp, ctx_ps, sums_ps)
                    for kp in range(NP):
                        if s < NQC:
                            emit_sim_pair(s, kp)
                        if ctx_live is not None and kp < NP - 1:
                            emit_ctx_pair(ctx_live[0], kp, ctx_live[1],
                                          ctx_live[2])
                    if ctx_live is not None:
                        emit_last_pair_and_norm(*ctx_live)
                        exp_pairs.pop(ctx_live[0])
                        ctx_live = None

    nc.compile()
    return nc


_PROGRAM = None


def _get_program():
    global _PROGRAM
    if _PROGRAM is None:
        _PROGRAM = _build_program()
    return _PROGRAM


def _prepare_in_maps(
    query_feats, key_feats, Wq, Wk, Wv,
    scale_q, bias_q, scale_k, bias_k, scale_v, bias_v,
):
    r = _round_fp32r
    f32 = np.float32
    qf_all = np.asarray(query_feats, f32).reshape(B, C, NK)
    kf_all = np.asarray(key_feats, f32).reshape(B, C, NK)

    wqT = r(np.ascontiguousarray(
        (np.asarray(scale_q, f32)[:, None] * np.asarray(Wq, f32)).T))
    wkT = r(np.ascontiguousarray(
        (np.asarray(scale_k, f32)[:, None] * np.asarray(Wk, f32)).T))
    wvT = np.zeros((C, C + 2), f32)
    wvT[:, :C] = r(np.ascontiguousarray(
        (np.asarray(scale_v, f32)[:, None] * np.asarray(Wv, f32)).T))
    bq2 = np.ascontiguousarray(np.asarray(bias_q, f32).reshape(CO, P).T)
    bk2 = np.ascontiguousarray(np.asarray(bias_k, f32).reshape(CO, P).T)
    bvb = np.zeros((P, C + 2), f32)
    bvb[:, :C] = r(np.asarray(bias_v, f32))[None, :]
    bvb[:, C] = 1.0
    ones = np.ones((1, P), f32)

    shared = dict(wqT=wqT, wkT=wkT, wvT=wvT, bq=bq2, bk=bk2,
                  bvb=bvb, ones=ones)
    in_maps = []
    for core in range(8):
        b, h = divmod(core, 2)
        in_maps.append(dict(
            qf=r(np.ascontiguousarray(qf_all[b][:, h * NQ:(h + 1) * NQ])),
            kf=r(np.ascontiguousarray(kf_all[b])),
            **shared,
        ))
    return in_maps


def run(inputs: dict, trace: bool = False):
    """Compile (cached) + run on 8 cores. Returns (output, BassKernelResults)."""
    nc = _get_program()
    in_maps = _prepare_in_maps(**inputs)
    res = run_bass_kernel_spmd(nc, in_maps, core_ids=list(range(8)), trace=trace)
    full = np.empty((B, C, NK), np.float32)
    for core in range(8):
        b, h = divmod(core, 2)
        full[b][:, h * NQ:(h + 1) * NQ] = res.results[core]["out"]
    return full.reshape(B, C, H, W), res


def kernel(**inputs) -> np.ndarray:
    return run(inputs)[0]



# revision 2
# speedup vs baseline: 1.0667x; 1.0667x over previous
"""Cross-attention Trainium2 Bass kernel.

Reference computation (per batch b):
    q = relu(scale_q * (Wq @ qf) + bias_q)          [C, Nq]
    k = relu(scale_k * (Wk @ kf) + bias_k)          [C, Nk]
    v = relu(scale_v * (Wv @ kf) + bias_v)          [C, Nk]
    sim  = q.T @ k / sqrt(C)                        [Nq, Nk]
    attn = softmax(sim, axis=-1)
    ctx  = v @ attn.T                               [C, Nq]

Sharding: 8 cores = 4 batches x 2 query halves (Nq 4096 -> 2048 per core).
Each core gets the full K/V for its batch (recomputed, cheap) and half the
query positions; output halves are concatenated (and transposed) on the host.

Device-side design (per core):
  - BN scale folded into the weights on the host; weights fed pre-transposed.
  - Projections and sim run in float32r (fp32 with 11-bit mantissa, full PE
    rate). DMA-fed matmul operands are pre-rounded on the host.
  - sim is computed transposed (k on partitions, q on free dim); softmax uses
    a constant shift instead of a row max: exp(sim/sqrt(C) - 4) (sim/sqrt(C)
    is bounded by ~|q||k|/16 << 88, so no overflow is possible). exp tiles are
    written in bf16.
  - ctx runs TRANSPOSED: stationary = exp tile [128k, 128q], moving =
    v^T [128k, 257] whose column 256 is an all-ones channel, so each PSUM
    accumulator [128q, 257] collects the 256 context channels AND the softmax
    denominator in the same pass -- no separate row-sum matmuls. A fresh
    128-col bf16 stationary per matmul loads in ~53ns (FWL) and hides under
    the 257-cycle moving stream, so the per-matmul cost is ~N cycles.
  - Normalization is per-partition (q on partitions): reciprocal_approx_fast
    on the denominator column [128,1] + one tensor_scalar multiply -- no
    cross-partition broadcast matmul, no 1-lane reciprocal.
  - Software pipeline: step s interleaves sim+exp for chunk s (16 k-pair
    tiles) with the 4 ctx q-tile accumulations (32 matmuls each, split in
    8-matmul quarters) consuming chunk s-1. PSUM: 4 banks sim double-buffer +
    4 banks ctx accumulators = 8.
  - Per-core output is [Nq, C] (q-major); the host transposes.
"""

import sys

for _p in ("/opt/trn_rl_repo", "/root/.axon_site/_ro/trn_rl_repo"):
    if _p not in sys.path:
        sys.path.insert(0, _p)

import numpy as np

import concourse.bacc as bacc
import concourse.mybir as mybir
import concourse.tile as tile
from concourse.bass_utils import run_bass_kernel_spmd

F32 = mybir.dt.float32
F32R = mybir.dt.float32r
BF16 = mybir.dt.bfloat16
AF = mybir.ActivationFunctionType

B, C, H, W = 4, 256, 64, 64
NK = H * W          # 4096 key positions per batch
NQ = NK // 2        # 2048 query positions per core
P = 128
CO = C // P         # 2 contraction subtiles
QC = 512            # query chunk (sim moving free dim)
NQC = NQ // QC      # 4 query chunks per core
KT = NK // P        # 32 key tiles
NP = KT // 2        # 16 key-pair tiles
QT = QC // P        # 4 q-subtiles per chunk (ctx accumulators)
EXP_SHIFT = -4.0    # exp(sim/sqrt(C) + EXP_SHIFT); sim/sqrt(C) observed in [0.5, 7.5]
SCALE = 1.0 / np.sqrt(C)


def _round_fp32r(x: np.ndarray) -> np.ndarray:
    """Round fp32 to fp32r (11-bit mantissa, RNE) as the PE datapath expects."""
    u = np.ascontiguousarray(x, dtype=np.float32).view(np.uint32)
    lsb = (u >> 12) & 1
    r = ((u + 0x7FF + lsb) & np.uint32(0xFFFFF000)).astype(np.uint32)
    return r.view(np.float32)


def _build_program():
    nc = bacc.Bacc("TRN2", target_bir_lowering=False, debug=False)

    qf = nc.dram_tensor("qf", [C, NQ], F32R, kind="ExternalInput").ap()
    kf = nc.dram_tensor("kf", [C, NK], F32R, kind="ExternalInput").ap()
    wqT = nc.dram_tensor("wqT", [C, C], F32R, kind="ExternalInput").ap()
    wkT = nc.dram_tensor("wkT", [C, C], F32R, kind="ExternalInput").ap()
    wvT = nc.dram_tensor("wvT", [C, C + 2], F32R, kind="ExternalInput").ap()
    bq = nc.dram_tensor("bq", [P, CO], F32, kind="ExternalInput").ap()
    bk = nc.dram_tensor("bk", [P, CO], F32, kind="ExternalInput").ap()
    bvb = nc.dram_tensor("bvb", [P, C + 2], F32, kind="ExternalInput").ap()
    out = nc.dram_tensor("out", [NQ, C], F32, kind="ExternalOutput").ap()
    out_t = out.rearrange("(g p) c -> p g c", p=P)   # [128, 16, 256]

    with tile.TileContext(nc) as tc:
        with (
            nc.allow_low_precision(reason="fp32r/bf16 matmul operands"),
            tc.tile_pool(name="consts", bufs=1) as consts,
            tc.tile_pool(name="persist", bufs=1) as persist,
        ):
            # ---- constants (issue order matters: the first projection only
            # needs the first half of wqT + bq + the first qf chunk, so those
            # go first and the remaining weights ride behind the qf/kf
            # streams) ----
            wqT_sb = consts.tile([P, CO, C], F32R, name="wqT_sb")
            wqT_t = wqT.rearrange("(co ci) o -> ci co o", ci=P)
            nc.gpsimd.dma_start(wqT_sb[:, :, :P], wqT_t[:, :, :P])
            nc.gpsimd.dma_start(wqT_sb[:, :, P:], wqT_t[:, :, P:])
            bq_sb = consts.tile([P, CO], F32, name="bq_sb")
            wkT_sb = consts.tile([P, CO, C], F32R, name="wkT_sb")
            wvT_sb = consts.tile([P, CO, C + 2], F32R, name="wvT_sb")
            bk_sb = consts.tile([P, CO], F32, name="bk_sb")
            bvb_sb = consts.tile([P, C + 2], F32, name="bvb_sb")
            b0_sb = consts.tile([P, 1], F32, name="b0_sb")
            nc.vector.memset(b0_sb[:], EXP_SHIFT)
            # dummy activation: pulls the ~1.3us LoadActFuncSet into the
            # initial DMA-wait window instead of blocking the first relu
            warm_sb = consts.tile([P, 1], F32, name="warm_sb")
            nc.scalar.activation(warm_sb[:], b0_sb[:], AF.Relu)

            # ---- persistent activations ----
            q_sb = persist.tile([P, CO, NQ], F32R, name="q_sb")
            k_sb = persist.tile([P, CO, NK], F32R, name="k_sb")
            vT_sb = persist.tile([P, KT, C + 2], BF16, name="vT_sb")

            # ---- projections (staging pool scoped so its SBUF is reused) ----
            with (
                tc.tile_pool(name="staging", bufs=1) as staging,
                tc.tile_pool(name="proj_ps", bufs=1, space="PSUM") as proj_ps,
            ):
                # Input DMA plan. Each dma_start costs ~650ns of serial SP
                # dispatch, so: few DMAs, a small first chunk so the first
                # matmul starts early, and strictly need-before order.
                qf_sb = staging.tile([P, CO, NQ], F32R, name="qf_sb")
                qf_t = qf.rearrange("(co ci) n -> ci co n", ci=P)
                kf_sb = staging.tile([P, CO, NK], F32R, name="kf_sb")
                kf_t = kf.rearrange("(co ci) n -> ci co n", ci=P)
                nc.gpsimd.dma_start(bq_sb[:], bq[:])
                nc.gpsimd.dma_start(wkT_sb[:], wkT.rearrange("(co ci) o -> ci co o", ci=P))
                nc.gpsimd.dma_start(bk_sb[:], bk[:])
                nc.gpsimd.dma_start(wvT_sb[:], wvT.rearrange("(co ci) o -> ci co o", ci=P))
                nc.gpsimd.dma_start(bvb_sb[:], bvb[:])
                nc.sync.dma_start(qf_sb[:, :, :QC], qf_t[:, :, :QC])
                nc.sync.dma_start(qf_sb[:, :, QC:], qf_t[:, :, QC:])
                nc.sync.dma_start(kf_sb[:, :, :2 * QC], kf_t[:, :, :2 * QC])
                nc.sync.dma_start(kf_sb[:, :, 2 * QC:5 * QC],
                                  kf_t[:, :, 2 * QC:5 * QC])
                nc.sync.dma_start(kf_sb[:, :, 5 * QC:], kf_t[:, :, 5 * QC:])

                def proj_iter(j, w_sb, bias_sb, dst, src_sb):
                    # one [*, QC] chunk of a q/k projection; relu+bias for
                    # oo=0 runs on ACT, oo=1 on DVE so neither engine
                    # rate-limits PE
                    for oo in range(CO):
                        ps = proj_ps.tile([P, QC], F32, tag="pj", bufs=2,
                                          name=f"ps_{j}_{oo}")
                        for co in range(CO):
                            nc.tensor.matmul(
                                ps[:],
                                w_sb[:, co, oo * P:(oo + 1) * P],
                                src_sb[:, co, j * QC:(j + 1) * QC],
                                start=(co == 0), stop=(co == CO - 1),
                            )
                        if oo == 0:
                            nc.scalar.activation(
                                dst[:, oo, j * QC:(j + 1) * QC], ps[:], AF.Relu,
                                bias=bias_sb[:, oo:oo + 1],
                            )
                        else:
                            nc.vector.tensor_scalar(
                                dst[:, oo, j * QC:(j + 1) * QC], ps[:],
                                bias_sb[:, oo:oo + 1], 0.0,
                                mybir.AluOpType.add, mybir.AluOpType.max,
                            )

                def vt_pair(kp):
                    # vT = relu(kf.T @ Wv'.T + bias_v): [n, o], n on partitions;
                    # column C is the ones channel (0-weight col + bias 1.0).
                    # bias_v varies along the free dim here, so it is added
                    # from a host-broadcast tile on DVE, then relu on ACT
                    # (writing bf16) -- no PE bias matmul needed.
                    psv = proj_ps.tile([P, 2, QC], F32, tag="pv", bufs=3,
                                       name=f"psv_{kp}")
                    for half in range(2):
                        kt = 2 * kp + half
                        for co in range(CO):
                            nc.tensor.matmul(
                                psv[:, half, :C + 2],
                                kf_sb[:, co, kt * P:(kt + 1) * P],
                                wvT_sb[:, co, :],
                                start=(co == 0), stop=(co == CO - 1),
                            )
                    vtmp = staging.tile([P, 2, C + 2], F32, tag="vtmp", bufs=3,
                                        name=f"vtmp_{kp}")
                    nc.vector.tensor_tensor(
                        vtmp[:], psv[:, :, :C + 2],
                        bvb_sb[:, None, :].to_broadcast((P, 2, C + 2)),
                        mybir.AluOpType.add,
                    )
                    nc.scalar.activation(
                        vT_sb[:, 2 * kp:2 * kp + 2, :], vtmp[:], AF.Relu)

                # q = relu(Wq' @ qf + bq): [o, n] with o on partitions.
                # j-major so each arriving qf/kf chunk is fully consumed at
                # once; the vT pairs for chunk j of kf ride along with proj-k
                # so PE work fills the relu latency.
                for j in range(NQ // QC):
                    proj_iter(j, wqT_sb, bq_sb, q_sb, qf_sb)
                for j in range(NK // QC):
                    proj_iter(j, wkT_sb, bk_sb, k_sb, kf_sb)
                    vt_pair(2 * j)
                    vt_pair(2 * j + 1)

            # ---- attention ----
            with (
                tc.tile_pool(name="expp", bufs=1) as expp,
                tc.tile_pool(name="outp", bufs=1) as outp,
                tc.tile_pool(name="attn_ps", bufs=1, space="PSUM") as attn_ps,
            ):
                # Software pipeline: step s emits sim+exp for chunk s
                # interleaved (at k-pair granularity) with the transposed ctx
                # matmuls consuming chunk s-1's exp tiles. Each ctx
                # accumulator covers one 128-query subtile and all 32 key
                # tiles; its 32 matmuls are issued in 8-matmul quarters after
                # successive sim pairs so PE never waits on ACT exp latency.
                e_pairs = {}    # qc -> list of 16 bf16 pair tiles
                out_tiles = {}  # qc -> [P, QT, C] staging tile for the chunk

                def emit_sim_pair(qc, kp):
                    qs = slice(qc * QC, (qc + 1) * QC)
                    ps = attn_ps.tile([P, 2, QC], F32, tag="sim", bufs=2,
                                      name=f"pss_{qc}_{kp}")
                    for half in range(2):
                        kt = 2 * kp + half
                        for co in range(CO):
                            nc.tensor.matmul(
                                ps[:, half, :],
                                k_sb[:, co, kt * P:(kt + 1) * P],
                                q_sb[:, co, qs],
                                start=(co == 0), stop=(co == CO - 1),
                            )
                    et = expp.tile([P, 2, QC], BF16, tag="expT", bufs=32,
                                   name=f"expT_{qc}_{kp}")
                    nc.scalar.activation(et[:], ps[:], AF.Exp,
                                         bias=b0_sb[:], scale=float(SCALE))
                    e_pairs.setdefault(qc, []).append(et)

                def emit_ctx_quarter(qc, qt, quarter, ctx_ps):
                    # 8 of the 32 accumulating matmuls for q-subtile qt:
                    # out[q, c] += e[k, q].T @ vT[k, c]; column C is the
                    # softmax denominator via vT's ones channel.
                    qoff = qt * P
                    for kt in range(quarter * 8, quarter * 8 + 8):
                        pair, half = divmod(kt, 2)
                        e = e_pairs[qc][pair][:, half, qoff:qoff + P]
                        nc.tensor.matmul(
                            ctx_ps[:, :C + 1],
                            e,
                            vT_sb[:, kt, :C + 1],
                            start=(kt == 0), stop=(kt == KT - 1),
                            skip_group_check=True,
                        )

                def emit_norm(qc, qt, ctx_ps):
                    recip = outp.tile([P, 1], F32, tag="recip", bufs=4,
                                      name=f"recip_{qc}_{qt}")
                    nc.vector.reciprocal_approx_fast(recip[:],
                                                     ctx_ps[:, C:C + 1])
                    ob = out_tiles[qc]
                    nc.vector.tensor_scalar_mul(ob[:, qt, :], ctx_ps[:, :C],
                                                recip[:])
                    if qt == QT - 1:
                        nc.sync.dma_start(out_t[:, qc * QT:(qc + 1) * QT, :],
                                          ob[:])

                for s in range(NQC + 1):
                    prev = s - 1
                    if prev >= 0:
                        out_tiles[prev] = outp.tile([P, QT, C], F32, tag="ob",
                                                    bufs=2, name=f"ob_{prev}")
                    ctx_ps = None
                    for kp in range(NP):
                        if s < NQC:
                            emit_sim_pair(s, kp)
                        if prev >= 0:
                            qt, quarter = divmod(kp, QT)
                            if quarter == 0:
                                ctx_ps = attn_ps.tile(
                                    [P, C + 1], F32, tag="ctx", bufs=4,
                                    name=f"psc_{prev}_{qt}")
                            emit_ctx_quarter(prev, qt, quarter, ctx_ps)
                            if quarter == QT - 1:
                                emit_norm(prev, qt, ctx_ps)
                    if prev >= 0:
                        e_pairs.pop(prev)

    nc.compile()
    return nc


_PROGRAM = None


def _get_program():
    global _PROGRAM
    if _PROGRAM is None:
        _PROGRAM = _build_program()
    return _PROGRAM


def _prepare_in_maps(
    query_feats, key_feats, Wq, Wk, Wv,
    scale_q, bias_q, scale_k, bias_k, scale_v, bias_v,
):
    r = _round_fp32r
    f32 = np.float32
    qf_all = np.asarray(query_feats, f32).reshape(B, C, NK)
    kf_all = np.asarray(key_feats, f32).reshape(B, C, NK)

    wqT = r(np.ascontiguousarray(
        (np.asarray(scale_q, f32)[:, None] * np.asarray(Wq, f32)).T))
    wkT = r(np.ascontiguousarray(
        (np.asarray(scale_k, f32)[:, None] * np.asarray(Wk, f32)).T))
    wvT = np.zeros((C, C + 2), f32)
    wvT[:, :C] = r(np.ascontiguousarray(
        (np.asarray(scale_v, f32)[:, None] * np.asarray(Wv, f32)).T))
    bq2 = np.ascontiguousarray(np.asarray(bias_q, f32).reshape(CO, P).T)
    bk2 = np.ascontiguousarray(np.asarray(bias_k, f32).reshape(CO, P).T)
    bvb = np.zeros((P, C + 2), f32)
    bvb[:, :C] = np.asarray(bias_v, f32)[None, :]
    bvb[:, C] = 1.0

    shared = dict(wqT=wqT, wkT=wkT, wvT=wvT, bq=bq2, bk=bk2, bvb=bvb)
    in_maps = []
    for core in range(8):
        b, h = divmod(core, 2)
        in_maps.append(dict(
            qf=r(np.ascontiguousarray(qf_all[b][:, h * NQ:(h + 1) * NQ])),
            kf=r(np.ascontiguousarray(kf_all[b])),
            **shared,
        ))
    return in_maps


def run(inputs: dict, trace: bool = False):
    """Compile (cached) + run on 8 cores. Returns (output, BassKernelResults)."""
    nc = _get_program()
    in_maps = _prepare_in_maps(**inputs)
    res = run_bass_kernel_spmd(nc, in_maps, core_ids=list(range(8)), trace=trace)
    full = np.empty((B, C, NK), np.float32)
    for core in range(8):
        b, h = divmod(core, 2)
        full[b][:, h * NQ:(h + 1) * NQ] = res.results[core]["out"].T
    return full.reshape(B, C, H, W), res


def kernel(**inputs) -> np.ndarray:
    return run(inputs)[0]


# revision 8
# speedup vs baseline: 1.2495x; 1.1714x over previous
"""Cross-attention Trainium2 Bass kernel.

Reference computation (per batch b):
    q = relu(scale_q * (Wq @ qf) + bias_q)          [C, Nq]
    k = relu(scale_k * (Wk @ kf) + bias_k)          [C, Nk]
    v = relu(scale_v * (Wv @ kf) + bias_v)          [C, Nk]
    sim  = q.T @ k / sqrt(C)                        [Nq, Nk]
    attn = softmax(sim, axis=-1)
    ctx  = v @ attn.T                               [C, Nq]

Sharding: 8 cores = 4 batches x 2 query halves (Nq 4096 -> 2048 per core).
Each core gets the full K/V for its batch (recomputed, cheap) and half the
query positions; output halves are concatenated (and transposed) on the host.

Device-side design (per core):
  - BN scale folded into the weights on the host; weights fed pre-transposed.
  - Projections and sim run in float32r (fp32 with 11-bit mantissa, full PE
    rate). DMA-fed matmul operands are pre-rounded on the host.
  - sim is computed transposed (k on partitions, q on free dim); softmax uses
    a constant shift instead of a row max: exp(sim/sqrt(C) - 4) (sim/sqrt(C)
    is bounded by ~|q||k|/16 << 88, so no overflow is possible). exp tiles are
    written in bf16.
  - ctx runs TRANSPOSED: stationary = exp tile [128k, 128q], moving =
    v^T [128k, 257] whose column 256 is an all-ones channel, so each PSUM
    accumulator [128q, 257] collects the 256 context channels AND the softmax
    denominator in the same pass -- no separate row-sum matmuls. A fresh
    128-col bf16 stationary per matmul loads in ~53ns (FWL) and hides under
    the 257-cycle moving stream, so the per-matmul cost is ~N cycles.
  - Normalization is per-partition (q on partitions): reciprocal_approx_fast
    on the denominator column [128,1] + one tensor_scalar multiply -- no
    cross-partition broadcast matmul, no 1-lane reciprocal.
  - Software pipeline: step s interleaves sim+exp for chunk s (16 k-pair
    tiles) with the 4 ctx q-tile accumulations (32 matmuls each, split in
    8-matmul quarters) consuming chunk s-1. PSUM: 4 banks sim double-buffer +
    4 banks ctx accumulators = 8.
  - Per-core output is [Nq, C] (q-major); the host transposes.
"""

import sys

for _p in ("/opt/trn_rl_repo", "/root/.axon_site/_ro/trn_rl_repo"):
    if _p not in sys.path:
        sys.path.insert(0, _p)

import numpy as np

import concourse.bacc as bacc
import concourse.mybir as mybir
import concourse.tile as tile
from concourse.bass_utils import run_bass_kernel_spmd

F32 = mybir.dt.float32
F32R = mybir.dt.float32r
BF16 = mybir.dt.bfloat16
AF = mybir.ActivationFunctionType

B, C, H, W = 4, 256, 64, 64
NK = H * W          # 4096 key positions per batch
NQ = NK // 2        # 2048 query positions per core
P = 128
CO = C // P         # 2 contraction subtiles
QC = 512            # query chunk (sim moving free dim)
NQC = NQ // QC      # 4 query chunks per core
KT = NK // P        # 32 key tiles
NP = KT // 2        # 16 key-pair tiles
QT = QC // P        # 4 q-subtiles per chunk (ctx accumulators)
EXP_SHIFT = -4.0    # exp(sim/sqrt(C) + EXP_SHIFT); sim/sqrt(C) observed in [0.5, 7.5]
SCALE = 1.0 / np.sqrt(C)


def _round_fp32r(x: np.ndarray) -> np.ndarray:
    """Round fp32 to fp32r (11-bit mantissa, RNE) as the PE datapath expects."""
    u = np.ascontiguousarray(x, dtype=np.float32).view(np.uint32)
    lsb = (u >> 12) & 1
    r = ((u + 0x7FF + lsb) & np.uint32(0xFFFFF000)).astype(np.uint32)
    return r.view(np.float32)


def _build_program():
    nc = bacc.Bacc("TRN2", target_bir_lowering=False, debug=False)

    qf = nc.dram_tensor("qf", [C, NQ], F32R, kind="ExternalInput").ap()
    kf = nc.dram_tensor("kf", [C, NK], F32R, kind="ExternalInput").ap()
    # weights pre-packed on the host into the exact SBUF per-partition layout
    # so each DMA is one contiguous >=2KB run per partition (big descriptors
    # stream much faster than the 512-1024B runs a strided rearrange produces)
    wqT = nc.dram_tensor("wqT", [P, CO * C], F32R, kind="ExternalInput").ap()
    wkT = nc.dram_tensor("wkT", [P, CO * C], F32R, kind="ExternalInput").ap()
    wvT = nc.dram_tensor("wvT", [P, CO * (C + 2)], F32R, kind="ExternalInput").ap()
    bq = nc.dram_tensor("bq", [P, CO], F32, kind="ExternalInput").ap()
    bk = nc.dram_tensor("bk", [P, CO], F32, kind="ExternalInput").ap()
    bvb = nc.dram_tensor("bvb", [P, C + 2], F32, kind="ExternalInput").ap()
    out = nc.dram_tensor("out", [NQ, C], F32, kind="ExternalOutput").ap()
    out_t = out.rearrange("(g p) c -> p g c", p=P)   # [128, 16, 256]

    with tile.TileContext(nc) as tc:
        with (
            nc.allow_low_precision(reason="fp32r/bf16 matmul operands"),
            tc.tile_pool(name="consts", bufs=1) as consts,
            tc.tile_pool(name="persist", bufs=1) as persist,
        ):
            # ---- constants (issue order matters: the first projection only
            # needs the first half of wqT + bq + the first qf chunk, so those
            # go first and the remaining weights ride behind the qf/kf
            # streams) ----
            wqT_sb = consts.tile([P, CO, C], F32R, name="wqT_sb")
            nc.gpsimd.dma_start(wqT_sb[:], wqT.rearrange("p (co o) -> p co o", co=CO))
            bq_sb = consts.tile([P, CO], F32, name="bq_sb")
            wkT_sb = consts.tile([P, CO, C], F32R, name="wkT_sb")
            wvT_sb = consts.tile([P, CO, C + 2], F32R, name="wvT_sb")
            bk_sb = consts.tile([P, CO], F32, name="bk_sb")
            bvb_sb = consts.tile([P, C + 2], F32, name="bvb_sb")
            b0_sb = consts.tile([P, 1], F32, name="b0_sb")
            nc.vector.memset(b0_sb[:], EXP_SHIFT)
            # dummy activation: pulls the ~1.3us LoadActFuncSet into the
            # initial DMA-wait window instead of blocking the first relu
            warm_sb = consts.tile([P, 1], F32, name="warm_sb")
            nc.scalar.activation(warm_sb[:], b0_sb[:], AF.Relu)

            # ---- persistent activations ----
            q_sb = persist.tile([P, CO, NQ], F32R, name="q_sb")
            k_sb = persist.tile([P, CO, NK], F32R, name="k_sb")
            vT_sb = persist.tile([P, KT, C + 2], BF16, name="vT_sb")

            # ---- projections (staging pool scoped so its SBUF is reused) ----
            with (
                tc.tile_pool(name="staging", bufs=1) as staging,
                tc.tile_pool(name="proj_ps", bufs=1, space="PSUM") as proj_ps,
            ):
                # Input DMA plan. Each dma_start costs ~650ns of serial SP
                # dispatch, so: few DMAs, a small first chunk so the first
                # matmul starts early, and strictly need-before order.
                qf_sb = staging.tile([P, CO, NQ], F32R, name="qf_sb")
                qf_t = qf.rearrange("(co ci) n -> ci co n", ci=P)
                kf_sb = staging.tile([P, CO, NK], F32R, name="kf_sb")
                kf_t = kf.rearrange("(co ci) n -> ci co n", ci=P)
                nc.gpsimd.dma_start(bq_sb[:], bq[:])
                nc.gpsimd.dma_start(wkT_sb[:], wkT.rearrange("p (co o) -> p co o", co=CO))
                nc.gpsimd.dma_start(bk_sb[:], bk[:])
                nc.gpsimd.dma_start(wvT_sb[:], wvT.rearrange("p (co o) -> p co o", co=CO))
                nc.gpsimd.dma_start(bvb_sb[:], bvb[:])
                # qf on the sync queue, kf on the scalar queue: the two input
                # streams run on independent DMA rings so k-projection data
                # is resident before the q-projections retire
                nc.sync.dma_start(qf_sb[:, :, :QC], qf_t[:, :, :QC])
                nc.sync.dma_start(qf_sb[:, :, QC:], qf_t[:, :, QC:])
                nc.scalar.dma_start(kf_sb[:, :, :2 * QC], kf_t[:, :, :2 * QC])
                nc.scalar.dma_start(kf_sb[:, :, 2 * QC:5 * QC],
                                    kf_t[:, :, 2 * QC:5 * QC])
                nc.scalar.dma_start(kf_sb[:, :, 5 * QC:], kf_t[:, :, 5 * QC:])

                def proj_iter(j, w_sb, bias_sb, dst, src_sb):
                    # one [*, QC] chunk of a q/k projection; relu+bias for
                    # oo=0 runs on ACT, oo=1 on DVE so neither engine
                    # rate-limits PE
                    for oo in range(CO):
                        ps = proj_ps.tile([P, QC], F32, tag="pj", bufs=2,
                                          name=f"ps_{j}_{oo}")
                        for co in range(CO):
                            nc.tensor.matmul(
                                ps[:],
                                w_sb[:, co, oo * P:(oo + 1) * P],
                                src_sb[:, co, j * QC:(j + 1) * QC],
                                start=(co == 0), stop=(co == CO - 1),
                            )
                        if oo == 0:
                            nc.scalar.activation(
                                dst[:, oo, j * QC:(j + 1) * QC], ps[:], AF.Relu,
                                bias=bias_sb[:, oo:oo + 1],
                            )
                        else:
                            nc.vector.tensor_scalar(
                                dst[:, oo, j * QC:(j + 1) * QC], ps[:],
                                bias_sb[:, oo:oo + 1], 0.0,
                                mybir.AluOpType.add, mybir.AluOpType.max,
                            )

                def vt_pair(kp):
                    # vT = relu(kf.T @ Wv'.T + bias_v): [n, o], n on partitions;
                    # column C is the ones channel (0-weight col + bias 1.0).
                    # bias_v varies along the free dim here, so it is added
                    # from a host-broadcast tile on DVE, then relu on ACT
                    # (writing bf16) -- no PE bias matmul needed.
                    psv = proj_ps.tile([P, 2, QC], F32, tag="pv", bufs=3,
                                       name=f"psv_{kp}")
                    for half in range(2):
                        kt = 2 * kp + half
                        for co in range(CO):
                            nc.tensor.matmul(
                                psv[:, half, :C + 2],
                                kf_sb[:, co, kt * P:(kt + 1) * P],
                                wvT_sb[:, co, :],
                                start=(co == 0), stop=(co == CO - 1),
                            )
                    vtmp = staging.tile([P, 2, C + 2], F32, tag="vtmp", bufs=3,
                                        name=f"vtmp_{kp}")
                    nc.vector.tensor_tensor(
                        vtmp[:], psv[:, :, :C + 2],
                        bvb_sb[:, None, :].to_broadcast((P, 2, C + 2)),
                        mybir.AluOpType.add,
                    )
                    nc.scalar.activation(
                        vT_sb[:, 2 * kp:2 * kp + 2, :], vtmp[:], AF.Relu)

                # q = relu(Wq' @ qf + bq): [o, n] with o on partitions.
                # j-major so each arriving qf/kf chunk is fully consumed at
                # once; the vT pairs for chunk j of kf ride along with proj-k
                # so PE work fills the relu latency.
                for j in range(NQ // QC):
                    proj_iter(j, wqT_sb, bq_sb, q_sb, qf_sb)
                for j in range(NK // QC):
                    proj_iter(j, wkT_sb, bk_sb, k_sb, kf_sb)
                    vt_pair(2 * j)
                    vt_pair(2 * j + 1)

            # ---- attention ----
            with (
                tc.tile_pool(name="expp", bufs=1) as expp,
                tc.tile_pool(name="outp", bufs=1) as outp,
                tc.tile_pool(name="attn_ps", bufs=1, space="PSUM") as attn_ps,
            ):
                # Software pipeline: step s emits sim+exp for chunk s
                # interleaved (at k-pair granularity) with the transposed ctx
                # matmuls consuming chunk s-1's exp tiles. Each ctx
                # accumulator covers one 128-query subtile and all 32 key
                # tiles; its 32 matmuls are issued in 8-matmul quarters after
                # successive sim pairs so PE never waits on ACT exp latency.
                e_pairs = {}    # qc -> list of 16 bf16 pair tiles
                out_tiles = {}  # qc -> [P, QT, C] staging tile for the chunk

                def emit_sim_pair(qc, kp):
                    qs = slice(qc * QC, (qc + 1) * QC)
                    ps = attn_ps.tile([P, 2, QC], F32, tag="sim", bufs=2,
                                      name=f"pss_{qc}_{kp}")
                    for half in range(2):
                        kt = 2 * kp + half
                        for co in range(CO):
                            nc.tensor.matmul(
                                ps[:, half, :],
                                k_sb[:, co, kt * P:(kt + 1) * P],
                                q_sb[:, co, qs],
                                start=(co == 0), stop=(co == CO - 1),
                            )
                    et = expp.tile([P, 2, QC], BF16, tag="expT", bufs=32,
                                   name=f"expT_{qc}_{kp}")
                    nc.scalar.activation(et[:], ps[:], AF.Exp,
                                         bias=b0_sb[:], scale=float(SCALE))
                    e_pairs.setdefault(qc, []).append(et)

                def emit_ctx_quarter(qc, qt, quarter, ctx_ps):
                    # 8 of the 32 accumulating matmuls for q-subtile qt:
                    # out[q, c] += e[k, q].T @ vT[k, c]; column C is the
                    # softmax denominator via vT's ones channel.
                    qoff = qt * P
                    for kt in range(quarter * 8, quarter * 8 + 8):
                        pair, half = divmod(kt, 2)
                        e = e_pairs[qc][pair][:, half, qoff:qoff + P]
                        nc.tensor.matmul(
                            ctx_ps[:, :C + 1],
                            e,
                            vT_sb[:, kt, :C + 1],
                            start=(kt == 0), stop=(kt == KT - 1),
                            skip_group_check=True,
                        )

                def emit_norm(qc, qt, ctx_ps):
                    recip = outp.tile([P, 1], F32, tag="recip", bufs=4,
                                      name=f"recip_{qc}_{qt}")
                    nc.vector.reciprocal_approx_fast(recip[:],
                                                     ctx_ps[:, C:C + 1])
                    ob = out_tiles[qc]
                    nc.vector.tensor_scalar_mul(ob[:, qt, :], ctx_ps[:, :C],
                                                recip[:])
                    # per-subtile output DMA so the last chunk's stores
                    # overlap its remaining ctx blocks instead of serializing
                    # into the kernel tail
                    nc.sync.dma_start(out_t[:, qc * QT + qt, :], ob[:, qt, :])

                for s in range(NQC + 1):
                    prev = s - 1
                    if prev >= 0:
                        out_tiles[prev] = outp.tile([P, QT, C], F32, tag="ob",
                                                    bufs=2, name=f"ob_{prev}")
                    ctx_ps = None
                    for kp in range(NP):
                        if s < NQC:
                            emit_sim_pair(s, kp)
                        if prev >= 0:
                            qt, quarter = divmod(kp, QT)
                            if quarter == 0:
                                ctx_ps = attn_ps.tile(
                                    [P, C + 1], F32, tag="ctx", bufs=4,
                                    name=f"psc_{prev}_{qt}")
                            emit_ctx_quarter(prev, qt, quarter, ctx_ps)
                            if quarter == QT - 1:
                                emit_norm(prev, qt, ctx_ps)
                    if prev >= 0:
                        e_pairs.pop(prev)

    nc.compile()
    return nc


_PROGRAM = None


def _get_program():
    global _PROGRAM
    if _PROGRAM is None:
        _PROGRAM = _build_program()
    return _PROGRAM


def _prepare_in_maps(
    query_feats, key_feats, Wq, Wk, Wv,
    scale_q, bias_q, scale_k, bias_k, scale_v, bias_v,
):
    r = _round_fp32r
    f32 = np.float32
    qf_all = np.asarray(query_feats, f32).reshape(B, C, NK)
    kf_all = np.asarray(key_feats, f32).reshape(B, C, NK)

    def pack_w(wT):
        # [C, M] weight -> [P, CO*M]: row ci holds the co=0 then co=1 slab,
        # matching the SBUF tile layout [ci][co][o] for contiguous DMA
        m = wT.shape[1]
        return np.ascontiguousarray(
            wT.reshape(CO, P, m).transpose(1, 0, 2).reshape(P, CO * m))

    wqT = pack_w(r(np.ascontiguousarray(
        (np.asarray(scale_q, f32)[:, None] * np.asarray(Wq, f32)).T)))
    wkT = pack_w(r(np.ascontiguousarray(
        (np.asarray(scale_k, f32)[:, None] * np.asarray(Wk, f32)).T)))
    wvT_2d = np.zeros((C, C + 2), f32)
    wvT_2d[:, :C] = r(np.ascontiguousarray(
        (np.asarray(scale_v, f32)[:, None] * np.asarray(Wv, f32)).T))
    wvT = pack_w(wvT_2d)
    bq2 = np.ascontiguousarray(np.asarray(bias_q, f32).reshape(CO, P).T)
    bk2 = np.ascontiguousarray(np.asarray(bias_k, f32).reshape(CO, P).T)
    bvb = np.zeros((P, C + 2), f32)
    bvb[:, :C] = np.asarray(bias_v, f32)[None, :]
    bvb[:, C] = 1.0

    shared = dict(wqT=wqT, wkT=wkT, wvT=wvT, bq=bq2, bk=bk2, bvb=bvb)
    in_maps = []
    for core in range(8):
        b, h = divmod(core, 2)
        in_maps.append(dict(
            qf=r(np.ascontiguousarray(qf_all[b][:, h * NQ:(h + 1) * NQ])),
            kf=r(np.ascontiguousarray(kf_all[b])),
            **shared,
        ))
    return in_maps


def run(inputs: dict, trace: bool = False):
    """Compile (cached) + run on 8 cores. Returns (output, BassKernelResults)."""
    nc = _get_program()
    in_maps = _prepare_in_maps(**inputs)
    res = run_bass_kernel_spmd(nc, in_maps, core_ids=list(range(8)), trace=trace)
    full = np.empty((B, C, NK), np.float32)
    for core in range(8):
        b, h = divmod(core, 2)
        full[b][:, h * NQ:(h + 1) * NQ] = res.results[core]["out"].T
    return full.reshape(B, C, H, W), res


def kernel(**inputs) -> np.ndarray:
    return run(inputs)[0]


# revision 10
# speedup vs baseline: 1.8566x; 1.4858x over previous
"""Cross-attention Trainium2 Bass kernel.

Reference computation (per batch b):
    q = relu(scale_q * (Wq @ qf) + bias_q)          [C, Nq]
    k = relu(scale_k * (Wk @ kf) + bias_k)          [C, Nk]
    v = relu(scale_v * (Wv @ kf) + bias_v)          [C, Nk]
    sim  = q.T @ k / sqrt(C)                        [Nq, Nk]
    attn = softmax(sim, axis=-1)
    ctx  = v @ attn.T                               [C, Nq]

Sharding: 8 cores = 4 batches x 2 query halves (Nq 4096 -> 2048 per core).
Each core gets the full K/V for its batch (recomputed, cheap) and half the
query positions; output halves are concatenated (and transposed) on the host.

Device-side design (per core):
  - BN scale folded into the weights on the host; weights fed pre-transposed
    and pre-packed into the SBUF per-partition layout so every weight DMA is
    one contiguous >=2KB run per partition.
  - Projections run in float32r; their outputs q/k/v are written as fp8e4m3.
    Softmax renormalization + averaging over 4096 keys washes the fp8
    quantization noise out (measured ~2e-3 relative Frobenius error vs the
    2e-2 budget).
  - sim is computed transposed (k on partitions, q on free dim) as ONE
    DoubleRow fp8 matmul per key tile: the [128c, 2co, 128k] stationary
    contracts all 256 channels in a single pass at 2 MACs/cell/cycle.
    Softmax uses a constant shift instead of a row max: exp(sim/sqrt(C) - 4)
    (sim/sqrt(C) is bounded by ~|q||k|/16 << 88, so no overflow); exp tiles
    are written as fp8e4m3 (values in [e^-3.5, e^3.5]).
  - ctx runs TRANSPOSED as DoubleRow fp8: stationary = exp pair [128k, 2kt,
    128q], moving = v^T pair [128k, 2kt, 257] whose column 256 is an all-ones
    channel, so each PSUM accumulator [128q, 257] collects the 256 context
    channels AND the softmax denominator in the same pass -- no separate
    row-sum matmuls, and each matmul covers TWO key tiles.
  - Normalization is per-partition (q on partitions): reciprocal_approx_fast
    on the denominator column [128,1] + one tensor_scalar multiply, then a
    per-subtile output DMA.
  - Flat software pipeline: the sim/exp stream runs at k-pair granularity;
    each ctx quadruplet (4 DoubleRow matmuls for one accumulator) issues 6
    pairs behind the exp that feeds it, so the fill/drain bubbles are ~6
    pairs instead of a full chunk, and PE work always covers the ACT exp
    latency.
  - Per-core output is [Nq, C] (q-major); the host transposes.
"""

import sys

for _p in ("/opt/trn_rl_repo", "/root/.axon_site/_ro/trn_rl_repo"):
    if _p not in sys.path:
        sys.path.insert(0, _p)

import numpy as np

import concourse.bacc as bacc
import concourse.mybir as mybir
import concourse.tile as tile
from concourse.bass_utils import run_bass_kernel_spmd

F32 = mybir.dt.float32
F32R = mybir.dt.float32r
FP8 = mybir.dt.float8e4
AF = mybir.ActivationFunctionType
DR = mybir.MatmulPerfMode.DoubleRow

B, C, H, W = 4, 256, 64, 64
NK = H * W          # 4096 key positions per batch
NQ = NK // 2        # 2048 query positions per core
P = 128
CO = 2              # contraction subtiles (C = 2*128)
QC = 512            # query chunk (sim moving free dim)
NQC = NQ // QC      # 4 query chunks per core
KT = NK // P        # 32 key tiles
NP = KT // 2        # 16 key-pair tiles per chunk
QT = QC // P        # 4 q-subtiles per chunk (ctx accumulators)
VF = 272            # vT free-dim pitch (>=258, 16B-aligned for DoubleRow)
LAG = 6             # ctx quadruplets trail the sim/exp stream by 6 pairs
EXP_SHIFT = -4.0    # exp(sim/sqrt(C) + EXP_SHIFT); sim/sqrt(C) observed in [0.5, 7.5]
SCALE = 1.0 / np.sqrt(C)


def _round_fp32r(x: np.ndarray) -> np.ndarray:
    """Round fp32 to fp32r (11-bit mantissa, RNE) as the PE datapath expects."""
    u = np.ascontiguousarray(x, dtype=np.float32).view(np.uint32)
    lsb = (u >> 12) & 1
    r = ((u + 0x7FF + lsb) & np.uint32(0xFFFFF000)).astype(np.uint32)
    return r.view(np.float32)


def _build_program():
    nc = bacc.Bacc("TRN2", target_bir_lowering=False, debug=False)

    qf = nc.dram_tensor("qf", [C, NQ], F32R, kind="ExternalInput").ap()
    kf = nc.dram_tensor("kf", [C, NK], F32R, kind="ExternalInput").ap()
    wqT = nc.dram_tensor("wqT", [P, CO * C], F32R, kind="ExternalInput").ap()
    wkT = nc.dram_tensor("wkT", [P, CO * C], F32R, kind="ExternalInput").ap()
    wvT = nc.dram_tensor("wvT", [P, CO * (C + 2)], F32R, kind="ExternalInput").ap()
    bq = nc.dram_tensor("bq", [P, CO], F32, kind="ExternalInput").ap()
    bk = nc.dram_tensor("bk", [P, CO], F32, kind="ExternalInput").ap()
    bvb = nc.dram_tensor("bvb", [P, C + 2], F32, kind="ExternalInput").ap()
    out = nc.dram_tensor("out", [NQ, C], F32, kind="ExternalOutput").ap()
    out_t = out.rearrange("(g p) c -> p g c", p=P)   # [128, 16, 256]

    with tile.TileContext(nc) as tc:
        with (
            nc.allow_low_precision(reason="fp32r/fp8 matmul operands"),
            tc.tile_pool(name="consts", bufs=1) as consts,
            tc.tile_pool(name="persist", bufs=1) as persist,
        ):
            # ---- constants (issue order matters: the first projection only
            # needs wqT + bq + the first qf chunk, so those go first and the
            # remaining weights ride behind the qf/kf streams) ----
            wqT_sb = consts.tile([P, CO, C], F32R, name="wqT_sb")
            nc.gpsimd.dma_start(wqT_sb[:], wqT.rearrange("p (co o) -> p co o", co=CO))
            bq_sb = consts.tile([P, CO], F32, name="bq_sb")
            wkT_sb = consts.tile([P, CO, C], F32R, name="wkT_sb")
            wvT_sb = consts.tile([P, CO, C + 2], F32R, name="wvT_sb")
            bk_sb = consts.tile([P, CO], F32, name="bk_sb")
            bvb_sb = consts.tile([P, C + 2], F32, name="bvb_sb")
            b0_sb = consts.tile([P, 1], F32, name="b0_sb")
            nc.vector.memset(b0_sb[:], EXP_SHIFT)
            # dummy activation: pulls the ~1.3us LoadActFuncSet into the
            # initial DMA-wait window instead of blocking the first relu
            warm_sb = consts.tile([P, 1], F32, name="warm_sb")
            nc.scalar.activation(warm_sb[:], b0_sb[:], AF.Relu)

            # ---- persistent activations (fp8) ----
            q_sb = persist.tile([P, CO, NQ], FP8, name="q_sb")
            k_sb = persist.tile([P, CO, NK], FP8, name="k_sb")
            vT_sb = persist.tile([P, KT, VF], FP8, name="vT_sb")

            # ---- projections (staging pool scoped so its SBUF is reused) ----
            with (
                tc.tile_pool(name="staging", bufs=1) as staging,
                tc.tile_pool(name="proj_ps", bufs=1, space="PSUM") as proj_ps,
            ):
                # Input DMA plan. Each dma_start costs ~650ns of serial SP
                # dispatch, so: few DMAs, a small first chunk so the first
                # matmul starts early, and strictly need-before order. qf and
                # kf share the sync queue so kf never contends with the
                # startup-critical qf/wqT transfers.
                qf_sb = staging.tile([P, CO, NQ], F32R, name="qf_sb")
                qf_t = qf.rearrange("(co ci) n -> ci co n", ci=P)
                kf_sb = staging.tile([P, CO, NK], F32R, name="kf_sb")
                kf_t = kf.rearrange("(co ci) n -> ci co n", ci=P)
                nc.gpsimd.dma_start(bq_sb[:], bq[:])
                nc.gpsimd.dma_start(wkT_sb[:], wkT.rearrange("p (co o) -> p co o", co=CO))
                nc.gpsimd.dma_start(bk_sb[:], bk[:])
                nc.gpsimd.dma_start(wvT_sb[:], wvT.rearrange("p (co o) -> p co o", co=CO))
                nc.gpsimd.dma_start(bvb_sb[:], bvb[:])
                nc.sync.dma_start(qf_sb[:, :, :QC], qf_t[:, :, :QC])
                nc.sync.dma_start(qf_sb[:, :, QC:], qf_t[:, :, QC:])
                nc.sync.dma_start(kf_sb[:, :, :2 * QC], kf_t[:, :, :2 * QC])
                nc.sync.dma_start(kf_sb[:, :, 2 * QC:5 * QC],
                                  kf_t[:, :, 2 * QC:5 * QC])
                nc.sync.dma_start(kf_sb[:, :, 5 * QC:], kf_t[:, :, 5 * QC:])

                def proj_iter(j, w_sb, bias_sb, dst, src_sb):
                    # one [*, QC] chunk of a q/k projection; relu+bias+fp8
                    # cast for oo=0 runs on ACT, oo=1 on DVE so neither
                    # engine rate-limits PE
                    for oo in range(CO):
                        ps = proj_ps.tile([P, QC], F32, tag="pj", bufs=2,
                                          name=f"ps_{j}_{oo}")
                        for co in range(CO):
                            nc.tensor.matmul(
                                ps[:],
                                w_sb[:, co, oo * P:(oo + 1) * P],
                                src_sb[:, co, j * QC:(j + 1) * QC],
                                start=(co == 0), stop=(co == CO - 1),
                            )
                        if oo == 0:
                            nc.scalar.activation(
                                dst[:, oo, j * QC:(j + 1) * QC], ps[:], AF.Relu,
                                bias=bias_sb[:, oo:oo + 1],
                            )
                        else:
                            nc.vector.tensor_scalar(
                                dst[:, oo, j * QC:(j + 1) * QC], ps[:],
                                bias_sb[:, oo:oo + 1], 0.0,
                                mybir.AluOpType.add, mybir.AluOpType.max,
                            )

                def vt_pair(kp):
                    # vT = relu(kf.T @ Wv'.T + bias_v): [n, o], n on partitions;
                    # column C is the ones channel (0-weight col + bias 1.0).
                    # bias_v varies along the free dim here, so it is added
                    # from a host-broadcast tile on DVE, then relu on ACT
                    # (writing fp8) -- no PE bias matmul needed.
                    psv = proj_ps.tile([P, 2, QC], F32, tag="pv", bufs=3,
                                       name=f"psv_{kp}")
                    for half in range(2):
                        kt = 2 * kp + half
                        for co in range(CO):
                            nc.tensor.matmul(
                                psv[:, half, :C + 2],
                                kf_sb[:, co, kt * P:(kt + 1) * P],
                                wvT_sb[:, co, :],
                                start=(co == 0), stop=(co == CO - 1),
                            )
                    vtmp = staging.tile([P, 2, C + 2], F32, tag="vtmp", bufs=3,
                                        name=f"vtmp_{kp}")
                    nc.vector.tensor_tensor(
                        vtmp[:], psv[:, :, :C + 2],
                        bvb_sb[:, None, :].to_broadcast((P, 2, C + 2)),
                        mybir.AluOpType.add,
                    )
                    nc.scalar.activation(
                        vT_sb[:, 2 * kp:2 * kp + 2, :C + 2], vtmp[:], AF.Relu)

                # q = relu(Wq' @ qf + bq): [o, n] with o on partitions.
                # j-major so each arriving qf/kf chunk is fully consumed at
                # once; the vT pairs for chunk j of kf ride along with proj-k
                # so PE work fills the relu latency.
                for j in range(NQ // QC):
                    proj_iter(j, wqT_sb, bq_sb, q_sb, qf_sb)
                for j in range(NK // QC):
                    proj_iter(j, wkT_sb, bk_sb, k_sb, kf_sb)
                    vt_pair(2 * j)
                    vt_pair(2 * j + 1)

            # ---- attention ----
            with (
                tc.tile_pool(name="expp", bufs=1) as expp,
                tc.tile_pool(name="outp", bufs=1) as outp,
                tc.tile_pool(name="attn_ps", bufs=1, space="PSUM") as attn_ps,
            ):
                e_pairs = {}    # qc -> list of 16 fp8 pair tiles
                out_tiles = {}  # qc -> [P, QT, C] staging tile
                ctx_tiles = {}  # (qc, qt) -> PSUM accumulator

                def emit_sim_pair(qc, kp):
                    # 2 DoubleRow matmuls: each contracts all 256 channels
                    qs = slice(qc * QC, (qc + 1) * QC)
                    ps = attn_ps.tile([P, 2, QC], F32, tag="sim", bufs=2,
                                      name=f"pss_{qc}_{kp}")
                    for half in range(2):
                        kt = 2 * kp + half
                        nc.tensor.matmul(
                            ps[:, half, :],
                            k_sb[:, :, kt * P:(kt + 1) * P],
                            q_sb[:, :, qs],
                            start=True, stop=True, perf_mode=DR,
                        )
                    et = expp.tile([P, 2, QC], FP8, tag="expT", bufs=32,
                                   name=f"expT_{qc}_{kp}")
                    nc.scalar.activation(et[:], ps[:], AF.Exp,
                                         bias=b0_sb[:], scale=float(SCALE))
                    e_pairs.setdefault(qc, []).append(et)

                def emit_ctx_job(qc, j):
                    # job j of chunk qc: 4 DoubleRow matmuls (key pairs
                    # quarter*4..quarter*4+4) into accumulator qt; col C of
                    # the moving vT pair is the ones channel -> denominator.
                    quarter, qt = divmod(j, QT)
                    if quarter == 0:
                        ctx_tiles[(qc, qt)] = attn_ps.tile(
                            [P, C + 1], F32, tag="ctx", bufs=4,
                            name=f"psc_{qc}_{qt}")
                    ctx_ps = ctx_tiles[(qc, qt)]
                    qoff = qt * P
                    for kp in range(quarter * 4, quarter * 4 + 4):
                        nc.tensor.matmul(
                            ctx_ps[:],
                            e_pairs[qc][kp][:, :, qoff:qoff + P],
                            vT_sb[:, 2 * kp:2 * kp + 2, :C + 1],
                            start=(kp == 0), stop=(kp == NP - 1),
                            perf_mode=DR, skip_group_check=True,
                        )
                    if quarter == QT - 1:
                        emit_norm(qc, qt)

                def emit_norm(qc, qt):
                    ctx_ps = ctx_tiles.pop((qc, qt))
                    recip = outp.tile([P, 1], F32, tag="recip", bufs=4,
                                      name=f"recip_{qc}_{qt}")
                    nc.vector.reciprocal_approx_fast(recip[:],
                                                     ctx_ps[:, C:C + 1])
                    ob = out_tiles[qc]
                    nc.vector.tensor_scalar_mul(ob[:, qt, :], ctx_ps[:, :C],
                                                recip[:])
                    nc.sync.dma_start(out_t[:, qc * QT + qt, :], ob[:, qt, :])

                # flat pipeline over global pair slots: slot t runs sim pair
                # (t//NP, t%NP) and ctx job t-LAG (quarter-major, 4 matmuls);
                # the last LAG slots drain the remaining ctx jobs
                TOT = NQC * NP
                for t in range(TOT + LAG):
                    if t < TOT:
                        qc, kp = divmod(t, NP)
                        if kp == 0:
                            out_tiles[qc] = outp.tile(
                                [P, QT, C], F32, tag="ob", bufs=2,
                                name=f"ob_{qc}")
                        emit_sim_pair(qc, kp)
                    tj = t - LAG
                    if tj >= 0:
                        emit_ctx_job(*divmod(tj, NP))

    nc.compile()
    return nc


_PROGRAM = None


def _get_program():
    global _PROGRAM
    if _PROGRAM is None:
        _PROGRAM = _build_program()
    return _PROGRAM


def _prepare_in_maps(
    query_feats, key_feats, Wq, Wk, Wv,
    scale_q, bias_q, scale_k, bias_k, scale_v, bias_v,
):
    r = _round_fp32r
    f32 = np.float32
    qf_all = np.asarray(query_feats, f32).reshape(B, C, NK)
    kf_all = np.asarray(key_feats, f32).reshape(B, C, NK)

    def pack_w(wT):
        # [C, M] weight -> [P, CO*M]: row ci holds the co=0 then co=1 slab,
        # matching the SBUF tile layout [ci][co][o] for contiguous DMA
        m = wT.shape[1]
        return np.ascontiguousarray(
            wT.reshape(CO, P, m).transpose(1, 0, 2).reshape(P, CO * m))

    wqT = pack_w(r(np.ascontiguousarray(
        (np.asarray(scale_q, f32)[:, None] * np.asarray(Wq, f32)).T)))
    wkT = pack_w(r(np.ascontiguousarray(
        (np.asarray(scale_k, f32)[:, None] * np.asarray(Wk, f32)).T)))
    wvT_2d = np.zeros((C, C + 2), f32)
    wvT_2d[:, :C] = r(np.ascontiguousarray(
        (np.asarray(scale_v, f32)[:, None] * np.asarray(Wv, f32)).T))
    wvT = pack_w(wvT_2d)
    bq2 = np.ascontiguousarray(np.asarray(bias_q, f32).reshape(CO, P).T)
    bk2 = np.ascontiguousarray(np.asarray(bias_k, f32).reshape(CO, P).T)
    bvb = np.zeros((P, C + 2), f32)
    bvb[:, :C] = np.asarray(bias_v, f32)[None, :]
    bvb[:, C] = 1.0

    shared = dict(wqT=wqT, wkT=wkT, wvT=wvT, bq=bq2, bk=bk2, bvb=bvb)
    in_maps = []
    for core in range(8):
        b, h = divmod(core, 2)
        in_maps.append(dict(
            qf=r(np.ascontiguousarray(qf_all[b][:, h * NQ:(h + 1) * NQ])),
            kf=r(np.ascontiguousarray(kf_all[b])),
            **shared,
        ))
    return in_maps


def run(inputs: dict, trace: bool = False):
    """Compile (cached) + run on 8 cores. Returns (output, BassKernelResults)."""
    nc = _get_program()
    in_maps = _prepare_in_maps(**inputs)
    res = run_bass_kernel_spmd(nc, in_maps, core_ids=list(range(8)), trace=trace)
    full = np.empty((B, C, NK), np.float32)
    for core in range(8):
        b, h = divmod(core, 2)
        full[b][:, h * NQ:(h + 1) * NQ] = res.results[core]["out"].T
    return full.reshape(B, C, H, W), res


def kernel(**inputs) -> np.ndarray:
    return run(inputs)[0]
